# revision 3
# baseline (speedup 1.0000x reference)
"""Trainium2 Bass kernel for the GRU-Extended-Kalman block.

Strategy:
  - Pure data parallel: batch 65536 sharded 8192 rows per NeuronCore (8 cores).
  - All on-chip activations are FEATURE-MAJOR (features on partitions, batch on
    the free axis). Host transposes inputs/outputs; the device does zero
    transposes and every matmul is lhsT(=W^T chunk) stationary, batch moving.
  - Matmuls run in float32r (full PE rate for moving dim >= 256, ~tf32
    precision); accumulation fp32 in PSUM.
  - Weights/biases are packed host-side into one [128, WCOLS] array, DMA'd to
    SBUF once, resident for the whole kernel.
"""

import numpy as np

DS, DM = 16, 12
QD, PD, SD = DS * DS, DS * DS, DM * DM     # 256, 256, 144
B = 65536
N_CORES = 8
ROWS = B // N_CORES                        # 8192
BT = 512                                   # batch tile (free-axis columns)
NT = ROWS // BT                            # 16 tiles per core


def split128(n):
    out = [128] * (n // 128)
    if n % 128:
        out.append(n % 128)
    return out


# ---------------------------------------------------------------------------
# Host-side weight packing plan (shape-only; values filled from params later)
# ---------------------------------------------------------------------------
class Packer:
    def __init__(self):
        self.cols = 0
        self.fills = []    # (col_off, row0, np-array-getter) filled later

    def block(self, k, m):
        """Reserve a [k, m] block; returns (off, k, m)."""
        off = self.cols
        self.cols += m
        return (off, k, m)


class LinearPlan:
    """One linear layer y = W x + b, x chunked by k_splits, y by m_splits."""

    def __init__(self, P, k_splits, m_splits):
        self.k_splits = k_splits
        self.m_splits = m_splits
        self.w = []      # [m][k] -> block
        self.b = []      # [m] -> block
        for ms in m_splits:
            self.w.append([P.block(ks, ms) for ks in k_splits])
            self.b.append(P.block(ms, 1))


class MLPPlan:
    def __init__(self, P, dims, first_k_splits=None):
        self.layers = []
        for l in range(len(dims) - 1):
            ks = first_k_splits if (l == 0 and first_k_splits) else split128(dims[l])
            self.layers.append(LinearPlan(P, ks, split128(dims[l + 1])))


class GRUPlan:
    def __init__(self, P, din, dh, x_splits):
        self.dh = dh
        self.m_splits = split128(dh)
        h_splits = split128(dh)
        # gates r, z: x-part and h-part accumulate into one psum
        self.rz = []
        for g in range(2):
            self.rz.append({
                'wx': [[P.block(ks, ms) for ks in x_splits] for ms in self.m_splits],
                'wh': [[P.block(ks, ms) for ks in h_splits] for ms in self.m_splits],
                'b': [P.block(ms, 1) for ms in self.m_splits],
            })
        # gate n: x-part and h-part separate
        self.nx = [[P.block(ks, ms) for ks in x_splits] for ms in self.m_splits]
        self.nh = [[P.block(ks, ms) for ks in h_splits] for ms in self.m_splits]
        self.bnx = [P.block(ms, 1) for ms in self.m_splits]
        self.bnh = [P.block(ms, 1) for ms in self.m_splits]


P_ = Packer()
PLAN = {
    'QI': MLPPlan(P_, [16, 32, 32, 32, 32]),
    'Q_GRU': GRUPlan(P_, 32, QD, split128(32)),
    'PI': MLPPlan(P_, [16, 32, 32, 32, 256]),
    'P_GRU': GRUPlan(P_, 512, PD, split128(256) + split128(256)),
    'PS': MLPPlan(P_, [256, 144, 144]),
    'SI': MLPPlan(P_, [24, 24, 24, 24, 144]),
    'S_GRU': GRUPlan(P_, 288, SD, split128(144) + split128(144)),
    'KG': MLPPlan(P_, [400, 192, 192, 192, 192],
                  first_k_splits=split128(144) + split128(256)),
    'SK': MLPPlan(P_, [336, 256, 256, 256, 256],
                  first_k_splits=split128(144) + split128(192)),
    'PU': MLPPlan(P_, [512, 256, 256, 256, 256],
                  first_k_splits=split128(256) + split128(256)),
}
WCOLS = P_.cols


def _fill_linear(W, lp, WT, bias):
    """WT: [din, dout] np, bias: [dout]."""
    ko = 0
    for ki, ks in enumerate(lp.k_splits):
        mo = 0
        for mi, ms in enumerate(lp.m_splits):
            off, k, m = lp.w[mi][ki]
            W[0:k, off:off + m] = WT[ko:ko + ks, mo:mo + ms]
            mo += ms
        ko += ks
    mo = 0
    for mi, ms in enumerate(lp.m_splits):
        off, k, m = lp.b[mi]
        W[0:k, off] = bias[mo:mo + ms]
        mo += ms


def _fill_gate(W, blocks, WT_g):
    ko = 0
    # blocks: [m][k]; WT_g: [din, dh_m_total] for this gate
    n_k = len(blocks[0])
    # derive k sizes from blocks
    for ki in range(n_k):
        mo = 0
        for mi in range(len(blocks)):
            off, k, m = blocks[mi][ki]
            W[0:k, off:off + m] = WT_g[ko:ko + k, mo:mo + m]
            mo += m
        ko += blocks[0][ki][1]


def _fill_bias(W, blocks, vec):
    mo = 0
    for off, k, m in blocks:
        W[0:k, off] = vec[mo:mo + k]
        mo += k


def pack_weights(params):
    W = np.zeros((128, WCOLS), dtype=np.float32)
    for name in ('QI', 'PI', 'SI', 'PS', 'KG', 'SK', 'PU'):
        mp = PLAN[name]
        for lp, (Wm, bm) in zip(mp.layers, params[name]):
            _fill_linear(W, lp, np.asarray(Wm).T.astype(np.float32),
                         np.asarray(bm).astype(np.float32))
    for name in ('Q_GRU', 'P_GRU', 'S_GRU'):
        gp = PLAN[name]
        Wih, Whh, bih, bhh = [np.asarray(a).astype(np.float32)
                              for a in params[name]]
        dh = gp.dh
        WihT, WhhT = Wih.T, Whh.T   # [din, 3dh], [dh, 3dh]
        for g in range(2):  # r, z
            _fill_gate(W, gp.rz[g]['wx'], WihT[:, g * dh:(g + 1) * dh])
            _fill_gate(W, gp.rz[g]['wh'], WhhT[:, g * dh:(g + 1) * dh])
            _fill_bias(W, gp.rz[g]['b'],
                       bih[g * dh:(g + 1) * dh] + bhh[g * dh:(g + 1) * dh])
        _fill_gate(W, gp.nx, WihT[:, 2 * dh:3 * dh])
        _fill_gate(W, gp.nh, WhhT[:, 2 * dh:3 * dh])
        _fill_bias(W, gp.bnx, bih[2 * dh:3 * dh])
        _fill_bias(W, gp.bnh, bhh[2 * dh:3 * dh])
    return W


# ---------------------------------------------------------------------------
# Bass program
# ---------------------------------------------------------------------------
_COMPILED = {}


def _build():
    import concourse.bacc as bacc
    import concourse.tile as tile
    import concourse.mybir as mybir

    F32 = mybir.dt.float32
    F32R = mybir.dt.float32r
    AF = mybir.ActivationFunctionType
    ALU = mybir.AluOpType

    nc = bacc.Bacc("TRN2", target_bir_lowering=False, debug=False,
                   num_devices=N_CORES)

    d_in = {}
    for nm, f in (('f12', 24), ('f3', 16), ('f4', 16), ('qh', QD), ('ph', PD),
                  ('sh', SD)):
        d_in[nm] = nc.dram_tensor(nm, (f, ROWS), F32R, kind="ExternalInput").ap()
    d_w = nc.dram_tensor("wpack", (128, WCOLS), F32R, kind="ExternalInput").ap()
    d_out = {}
    for nm, f in (('kg', 192), ('q', QD), ('p', PD), ('s', SD)):
        d_out[nm] = nc.dram_tensor(nm, (f, ROWS), F32R, kind="ExternalOutput").ap()

    with tile.TileContext(nc) as tc:
        import contextlib
        ctx = contextlib.ExitStack()
        with ctx:
            wpool = ctx.enter_context(tc.tile_pool(name="w", bufs=1))
            io = ctx.enter_context(tc.tile_pool(name="io", bufs=2))
            act = ctx.enter_context(tc.tile_pool(name="act", bufs=1))
            act2 = ctx.enter_context(tc.tile_pool(name="act2", bufs=2))
            pspool = ctx.enter_context(tc.tile_pool(name="ps", bufs=6,
                                                    space="PSUM"))

            wsb = wpool.tile([128, WCOLS], F32R, tag="wsb")
            n_wdma = 4
            wc = (WCOLS + n_wdma - 1) // n_wdma
            for i in range(n_wdma):
                a, b = i * wc, min(WCOLS, (i + 1) * wc)
                nc.sync.dma_start(wsb[:, a:b], d_w[:, a:b])

            def WB(blk):
                off, k, m = blk
                return wsb[0:k, off:off + m]

            def BB(blk):
                off, k, m = blk
                return wsb[0:k, off:off + 1].bitcast(F32)

            def run_mlp(name, x_chunks, engine, t):
                """x_chunks: list of (ap, size). Returns output chunk list."""
                mp = PLAN[name]
                cur = x_chunks
                n_l = len(mp.layers)
                for l, lp in enumerate(mp.layers):
                    relu = l < n_l - 1
                    pool = act2 if name in ('QI', 'PI', 'SI') else act
                    ot = pool.tile([128, BT * len(lp.m_splits)], F32R,
                                   tag=f"{name}_h{l % 2}")
                    nxt = []
                    for mi, ms in enumerate(lp.m_splits):
                        ps = pspool.tile([128, BT], F32, tag="ps")
                        n_k = len(lp.k_splits)
                        assert n_k == len(cur), (name, l, n_k, len(cur))
                        for ki, (xap, ksz) in enumerate(cur):
                            nc.tensor.matmul(ps[0:ms, :], WB(lp.w[mi][ki]),
                                             xap, start=(ki == 0),
                                             stop=(ki == n_k - 1))
                        oap = ot[0:ms, mi * BT:(mi + 1) * BT]
                        if engine == 'act':
                            nc.scalar.activation(
                                oap, ps[0:ms, :],
                                AF.Relu if relu else AF.Identity,
                                bias=BB(lp.b[mi]))
                        else:
                            if relu:
                                nc.vector.tensor_scalar(
                                    oap, ps[0:ms, :], BB(lp.b[mi]), 0.0,
                                    op0=ALU.add, op1=ALU.max)
                            else:
                                nc.vector.tensor_scalar(
                                    oap, ps[0:ms, :], BB(lp.b[mi]), None,
                                    op0=ALU.add)
                        nxt.append((oap, ms))
                    cur = nxt
                return cur

            def run_gru(name, x_chunks, h_chunks, out_tag, t):
                gp = PLAN[name]
                hout = act.tile([128, BT * len(gp.m_splits)], F32R,
                                tag=out_tag)
                outs = []
                for mi, ms in enumerate(gp.m_splits):
                    mc = slice(mi * BT, (mi + 1) * BT)
                    # --- gates r and z: accumulate h-part then x-part
                    rz = []
                    for g in range(2):
                        ps = pspool.tile([128, BT], F32, tag="ps")
                        blks = ([(gp.rz[g]['wh'][mi][ki], hap)
                                 for ki, (hap, _) in enumerate(h_chunks)] +
                                [(gp.rz[g]['wx'][mi][ki], xap)
                                 for ki, (xap, _) in enumerate(x_chunks)])
                        for ii, (wb, rhs) in enumerate(blks):
                            nc.tensor.matmul(ps[0:ms, :], WB(wb), rhs,
                                             start=(ii == 0),
                                             stop=(ii == len(blks) - 1))
                        gt = act2.tile([128, BT], F32, tag=f"g{'rz'[g]}")
                        nc.scalar.activation(gt[0:ms, :], ps[0:ms, :],
                                             AF.Sigmoid,
                                             bias=BB(gp.rz[g]['b'][mi]))
                        rz.append(gt)
                    r_t, z_t = rz
                    # --- gate n
                    ps_hn = pspool.tile([128, BT], F32, tag="ps")
                    for ki, (hap, _) in enumerate(h_chunks):
                        nc.tensor.matmul(ps_hn[0:ms, :], WB(gp.nh[mi][ki]),
                                         hap, start=(ki == 0),
                                         stop=(ki == len(h_chunks) - 1))
                    ps_in = pspool.tile([128, BT], F32, tag="ps")
                    for ki, (xap, _) in enumerate(x_chunks):
                        nc.tensor.matmul(ps_in[0:ms, :], WB(gp.nx[mi][ki]),
                                         xap, start=(ki == 0),
                                         stop=(ki == len(x_chunks) - 1))
                    sc = act2.tile([128, BT], F32, tag="gsc")
                    # rhn = (hn + bhh_n) * r
                    nc.vector.scalar_tensor_tensor(
                        sc[0:ms, :], ps_hn[0:ms, :], BB(gp.bnh[mi]),
                        r_t[0:ms, :], op0=ALU.add, op1=ALU.mult)
                    # npre = (inn + bih_n) + rhn
                    nc.vector.scalar_tensor_tensor(
                        sc[0:ms, :], ps_in[0:ms, :], BB(gp.bnx[mi]),
                        sc[0:ms, :], op0=ALU.add, op1=ALU.add)
                    nt = act2.tile([128, BT], F32, tag="gn")
                    nc.scalar.activation(nt[0:ms, :], sc[0:ms, :], AF.Tanh)
                    # h' = n + z*(h - n)
                    hap = h_chunks[mi][0]
                    nc.vector.tensor_sub(sc[0:ms, :], hap, nt[0:ms, :])
                    nc.vector.tensor_mul(sc[0:ms, :], z_t[0:ms, :],
                                         sc[0:ms, :])
                    oap = hout[0:ms, mc]
                    nc.vector.tensor_add(oap, sc[0:ms, :], nt[0:ms, :])
                    outs.append((oap, ms))
                return outs

            def load_fm(nm, f, t, tag):
                sp = split128(f)
                tl = io.tile([128, BT * len(sp)], F32R, tag=tag)
                chunks = []
                fo = 0
                for ci, cs in enumerate(sp):
                    ap = tl[0:cs, ci * BT:(ci + 1) * BT]
                    nc.sync.dma_start(
                        ap, d_in[nm][fo:fo + cs, t * BT:(t + 1) * BT])
                    chunks.append((ap, cs))
                    fo += cs
                return chunks

            def store_fm(nm, chunks, t):
                fo = 0
                for ap, cs in chunks:
                    nc.sync.dma_start(
                        d_out[nm][fo:fo + cs, t * BT:(t + 1) * BT], ap)
                    fo += cs

            for t in range(NT):
                f12 = load_fm('f12', 24, t, 'f12')
                f3 = load_fm('f3', 16, t, 'f3')
                f4 = load_fm('f4', 16, t, 'f4')
                qh = load_fm('qh', QD, t, 'qh')
                ph = load_fm('ph', PD, t, 'ph')
                sh = load_fm('sh', SD, t, 'sh')

                qinp = run_mlp('QI', f4, 'act', t)
                pinp = run_mlp('PI', f3, 'act', t)
                sinp = run_mlp('SI', f12, 'act', t)

                q = run_gru('Q_GRU', qinp, qh, 'qout', t)
                store_fm('q', q, t)

                p1 = run_gru('P_GRU', q + pinp, ph, 'pout', t)

                psx = run_mlp('PS', p1, 'dve', t)
                s = run_gru('S_GRU', sinp + psx, sh, 'sout', t)
                store_fm('s', s, t)

                kg = run_mlp('KG', s + p1, 'dve', t)
                store_fm('kg', kg, t)

                sk = run_mlp('SK', s + kg, 'act', t)
                p2 = run_mlp('PU', p1 + sk, 'dve', t)
                store_fm('p', p2, t)

    nc.compile()
    return nc


def _get_nc():
    if 'nc' not in _COMPILED:
        _COMPILED['nc'] = _build()
    return _COMPILED['nc']


# ---------------------------------------------------------------------------
# Host entry point
# ---------------------------------------------------------------------------
def kernel(F1, F2, F3, F4, Q_GRU_HIDDEN, P_GRU_HIDDEN, S_GRU_HIDDEN, params,
           _want_results_obj=False, _trace=False):
    from concourse.bass_utils import run_bass_kernel_spmd

    F1 = np.asarray(F1, dtype=np.float32)
    F2 = np.asarray(F2, dtype=np.float32)
    F3 = np.asarray(F3, dtype=np.float32)
    F4 = np.asarray(F4, dtype=np.float32)
    QH = np.asarray(Q_GRU_HIDDEN, dtype=np.float32)
    PH = np.asarray(P_GRU_HIDDEN, dtype=np.float32)
    SH = np.asarray(S_GRU_HIDDEN, dtype=np.float32)

    f12T = np.empty((24, B), dtype=np.float32)
    f12T[0:12] = F1.T
    f12T[12:24] = F2.T
    f3T = np.ascontiguousarray(F3.T)
    f4T = np.ascontiguousarray(F4.T)
    qhT = np.ascontiguousarray(QH.T)
    phT = np.ascontiguousarray(PH.T)
    shT = np.ascontiguousarray(SH.T)

    W = pack_weights(params)

    nc = _get_nc()
    in_maps = []
    for c in range(N_CORES):
        sl = slice(c * ROWS, (c + 1) * ROWS)
        in_maps.append({
            'f12': np.ascontiguousarray(f12T[:, sl]),
            'f3': np.ascontiguousarray(f3T[:, sl]),
            'f4': np.ascontiguousarray(f4T[:, sl]),
            'qh': np.ascontiguousarray(qhT[:, sl]),
            'ph': np.ascontiguousarray(phT[:, sl]),
            'sh': np.ascontiguousarray(shT[:, sl]),
            'wpack': W,
        })
    res = run_bass_kernel_spmd(nc, in_maps, core_ids=list(range(N_CORES)),
                               trace=_trace)

    kgT = np.empty((192, B), dtype=np.float32)
    qT = np.empty((QD, B), dtype=np.float32)
    pT = np.empty((PD, B), dtype=np.float32)
    sT = np.empty((SD, B), dtype=np.float32)
    for c in range(N_CORES):
        sl = slice(c * ROWS, (c + 1) * ROWS)
        kgT[:, sl] = res.results[c]['kg']
        qT[:, sl] = res.results[c]['q']
        pT[:, sl] = res.results[c]['p']
        sT[:, sl] = res.results[c]['s']

    out = (np.ascontiguousarray(kgT.T).reshape(B, DS, DM),
           np.ascontiguousarray(qT.T),
           np.ascontiguousarray(pT.T),
           np.ascontiguousarray(sT.T))
    if _want_results_obj:
        return out, res
    return out


# revision 4
# speedup vs baseline: 1.0360x; 1.0360x over previous
"""Trainium2 Bass kernel for the GRU-Extended-Kalman block.

Strategy:
  - Pure data parallel: batch 65536 sharded 8192 rows per NeuronCore (8 cores).
  - All on-chip activations are FEATURE-MAJOR (features on partitions, batch on
    the free axis). Host transposes inputs/outputs; the device does zero
    transposes and every matmul is lhsT(=W^T chunk) stationary, batch moving.
  - Matmuls in bf16 (weights + activations), fp32 PSUM accumulate. GRU gate
    math and all outputs stay fp32; tensors that feed later matmuls are cast
    to bf16 (by the producing op, or by a GpSimd copy for dual-use tensors).
  - Weights/biases are packed host-side, DMA'd to SBUF once, resident.
"""

import numpy as np

DS, DM = 16, 12
QD, PD, SD = DS * DS, DS * DS, DM * DM     # 256, 256, 144
B = 65536
N_CORES = 8
ROWS = B // N_CORES                        # 8192
BT = 512                                   # batch tile (free-axis columns)
NT = ROWS // BT                            # 16 tiles per core


def split128(n):
    out = [128] * (n // 128)
    if n % 128:
        out.append(n % 128)
    return out


# ---------------------------------------------------------------------------
# Host-side weight packing plan (shape-only; values filled from params later)
# ---------------------------------------------------------------------------
class Packer:
    def __init__(self):
        self.cols = 0

    def block(self, k, m):
        off = self.cols
        self.cols += m
        return (off, k, m)


class LinearPlan:
    def __init__(self, PW, PB, k_splits, m_splits):
        self.k_splits = k_splits
        self.m_splits = m_splits
        self.w = [[PW.block(ks, ms) for ks in k_splits] for ms in m_splits]
        self.b = [PB.block(ms, 1) for ms in m_splits]


class MLPPlan:
    def __init__(self, PW, PB, dims, first_k_splits=None):
        self.layers = []
        for l in range(len(dims) - 1):
            ks = first_k_splits if (l == 0 and first_k_splits) else split128(dims[l])
            self.layers.append(LinearPlan(PW, PB, ks, split128(dims[l + 1])))


class GRUPlan:
    def __init__(self, PW, PB, din, dh, x_splits):
        self.dh = dh
        self.m_splits = split128(dh)
        h_splits = split128(dh)
        self.rz = []
        for g in range(2):
            self.rz.append({
                'wx': [[PW.block(ks, ms) for ks in x_splits] for ms in self.m_splits],
                'wh': [[PW.block(ks, ms) for ks in h_splits] for ms in self.m_splits],
                'b': [PB.block(ms, 1) for ms in self.m_splits],
            })
        self.nx = [[PW.block(ks, ms) for ks in x_splits] for ms in self.m_splits]
        self.nh = [[PW.block(ks, ms) for ks in h_splits] for ms in self.m_splits]
        self.bnx = [PB.block(ms, 1) for ms in self.m_splits]
        self.bnh = [PB.block(ms, 1) for ms in self.m_splits]


P_W = Packer()
P_B = Packer()
PLAN = {
    'QI': MLPPlan(P_W, P_B, [16, 32, 32, 32, 32]),
    'Q_GRU': GRUPlan(P_W, P_B, 32, QD, split128(32)),
    'PI': MLPPlan(P_W, P_B, [16, 32, 32, 32, 256]),
    'P_GRU': GRUPlan(P_W, P_B, 512, PD, split128(256) + split128(256)),
    'PS': MLPPlan(P_W, P_B, [256, 144, 144]),
    'SI': MLPPlan(P_W, P_B, [24, 24, 24, 24, 144]),
    'S_GRU': GRUPlan(P_W, P_B, 288, SD, split128(144) + split128(144)),
    'KG': MLPPlan(P_W, P_B, [400, 192, 192, 192, 192],
                  first_k_splits=split128(144) + split128(256)),
    'SK': MLPPlan(P_W, P_B, [336, 256, 256, 256, 256],
                  first_k_splits=split128(144) + split128(192)),
    'PU': MLPPlan(P_W, P_B, [512, 256, 256, 256, 256],
                  first_k_splits=split128(256) + split128(256)),
}
WCOLS = P_W.cols
BCOLS = P_B.cols


def _fill_linear(W, Bv, lp, WT, bias):
    ko = 0
    for ki, ks in enumerate(lp.k_splits):
        mo = 0
        for mi, ms in enumerate(lp.m_splits):
            off, k, m = lp.w[mi][ki]
            W[0:k, off:off + m] = WT[ko:ko + ks, mo:mo + ms]
            mo += ms
        ko += ks
    mo = 0
    for mi, ms in enumerate(lp.m_splits):
        off, k, m = lp.b[mi]
        Bv[0:k, off] = bias[mo:mo + ms]
        mo += ms


def _fill_gate(W, blocks, WT_g):
    ko = 0
    for ki in range(len(blocks[0])):
        mo = 0
        for mi in range(len(blocks)):
            off, k, m = blocks[mi][ki]
            W[0:k, off:off + m] = WT_g[ko:ko + k, mo:mo + m]
            mo += m
        ko += blocks[0][ki][1]


def _fill_bias(Bv, blocks, vec):
    mo = 0
    for off, k, m in blocks:
        Bv[0:k, off] = vec[mo:mo + k]
        mo += k


def pack_weights(params):
    import ml_dtypes
    W = np.zeros((128, WCOLS), dtype=np.float32)
    Bv = np.zeros((128, BCOLS), dtype=np.float32)
    for name in ('QI', 'PI', 'SI', 'PS', 'KG', 'SK', 'PU'):
        mp = PLAN[name]
        for lp, (Wm, bm) in zip(mp.layers, params[name]):
            _fill_linear(W, Bv, lp, np.asarray(Wm).T.astype(np.float32),
                         np.asarray(bm).astype(np.float32))
    for name in ('Q_GRU', 'P_GRU', 'S_GRU'):
        gp = PLAN[name]
        Wih, Whh, bih, bhh = [np.asarray(a).astype(np.float32)
                              for a in params[name]]
        dh = gp.dh
        WihT, WhhT = Wih.T, Whh.T
        for g in range(2):
            _fill_gate(W, gp.rz[g]['wx'], WihT[:, g * dh:(g + 1) * dh])
            _fill_gate(W, gp.rz[g]['wh'], WhhT[:, g * dh:(g + 1) * dh])
            _fill_bias(Bv, gp.rz[g]['b'],
                       bih[g * dh:(g + 1) * dh] + bhh[g * dh:(g + 1) * dh])
        _fill_gate(W, gp.nx, WihT[:, 2 * dh:3 * dh])
        _fill_gate(W, gp.nh, WhhT[:, 2 * dh:3 * dh])
        _fill_bias(Bv, gp.bnx, bih[2 * dh:3 * dh])
        _fill_bias(Bv, gp.bnh, bhh[2 * dh:3 * dh])
    return W.astype(ml_dtypes.bfloat16), Bv


# ---------------------------------------------------------------------------
# Bass program
# ---------------------------------------------------------------------------
_COMPILED = {}


def _build():
    import contextlib
    import concourse.bacc as bacc
    import concourse.tile as tile
    import concourse.mybir as mybir

    F32 = mybir.dt.float32
    BF16 = mybir.dt.bfloat16
    AF = mybir.ActivationFunctionType
    ALU = mybir.AluOpType

    nc = bacc.Bacc("TRN2", target_bir_lowering=False, debug=False,
                   num_devices=N_CORES)

    d_in = {}
    for nm, f in (('f12', 24), ('f3', 16), ('f4', 16), ('qh', QD), ('ph', PD),
                  ('sh', SD)):
        d_in[nm] = nc.dram_tensor(nm, (f, ROWS), BF16, kind="ExternalInput").ap()
    d_w = nc.dram_tensor("wpack", (128, WCOLS), BF16, kind="ExternalInput").ap()
    d_b = nc.dram_tensor("bpack", (128, BCOLS), F32, kind="ExternalInput").ap()
    d_out = {}
    for nm, f in (('kg', 192), ('q', QD), ('p', PD), ('s', SD)):
        d_out[nm] = nc.dram_tensor(nm, (f, ROWS), F32, kind="ExternalOutput").ap()

    with tile.TileContext(nc) as tc:
        ctx = contextlib.ExitStack()
        with ctx:
            wpool = ctx.enter_context(tc.tile_pool(name="w", bufs=1))
            io = ctx.enter_context(tc.tile_pool(name="io", bufs=3))
            act = ctx.enter_context(tc.tile_pool(name="act", bufs=2))
            act2 = ctx.enter_context(tc.tile_pool(name="act2", bufs=2))
            pspool = ctx.enter_context(tc.tile_pool(name="ps", bufs=6,
                                                    space="PSUM"))

            wsb = wpool.tile([128, WCOLS], BF16, tag="wsb")
            n_wdma = 4
            wc = (WCOLS + n_wdma - 1) // n_wdma
            for i in range(n_wdma):
                a, b = i * wc, min(WCOLS, (i + 1) * wc)
                nc.sync.dma_start(wsb[:, a:b], d_w[:, a:b])
            bsb = wpool.tile([128, BCOLS], F32, tag="bsb")
            nc.sync.dma_start(bsb[:, :], d_b[:, :])

            def WB(blk):
                off, k, m = blk
                return wsb[0:k, off:off + m]

            def BB(blk):
                off, k, m = blk
                return bsb[0:k, off:off + 1]

            def run_mlp(name, x_chunks, engine, final_f32=False):
                mp = PLAN[name]
                cur = x_chunks
                n_l = len(mp.layers)
                for l, lp in enumerate(mp.layers):
                    relu = l < n_l - 1
                    odt = F32 if (final_f32 and not relu) else BF16
                    pool = act2 if name in ('QI', 'PI', 'SI') else act
                    ot = pool.tile([128, BT * len(lp.m_splits)], odt,
                                   tag=f"{name}_h{l % 2}")
                    nxt = []
                    for mi, ms in enumerate(lp.m_splits):
                        ps = pspool.tile([128, BT], F32, tag="ps")
                        n_k = len(lp.k_splits)
                        for ki, (xap, ksz) in enumerate(cur):
                            nc.tensor.matmul(ps[0:ms, :], WB(lp.w[mi][ki]),
                                             xap, start=(ki == 0),
                                             stop=(ki == n_k - 1))
                        oap = ot[0:ms, mi * BT:(mi + 1) * BT]
                        if engine == 'act':
                            nc.scalar.activation(
                                oap, ps[0:ms, :],
                                AF.Relu if relu else AF.Identity,
                                bias=BB(lp.b[mi]))
                        else:
                            if relu:
                                nc.vector.tensor_scalar(
                                    oap, ps[0:ms, :], BB(lp.b[mi]), 0.0,
                                    op0=ALU.add, op1=ALU.max)
                            else:
                                nc.vector.tensor_scalar(
                                    oap, ps[0:ms, :], BB(lp.b[mi]), None,
                                    op0=ALU.add)
                        nxt.append((oap, ms))
                    cur = nxt
                return cur

            def run_gru(name, x_chunks, h_tile, out_tag, out_dt):
                """h_tile: io tile [128, n_chunks*BT] bf16. Returns (chunks,
                wide_tile)."""
                gp = PLAN[name]
                nch = len(gp.m_splits)
                W_ = BT * nch
                hout = act.tile([128, W_], out_dt, tag=out_tag)
                r_t = act2.tile([128, W_], F32, tag="g_r")
                z_t = act2.tile([128, W_], F32, tag="g_z")
                n_t = act2.tile([128, W_], F32, tag="g_n")
                sc = act2.tile([128, W_], F32, tag="g_sc")
                for mi, ms in enumerate(gp.m_splits):
                    mc = slice(mi * BT, (mi + 1) * BT)
                    for g, gt in ((0, r_t), (1, z_t)):
                        ps = pspool.tile([128, BT], F32, tag="ps")
                        blks = ([(gp.rz[g]['wh'][mi][ki], h_tile[0:ksz, ki * BT:ki * BT + BT])
                                 for ki, ksz in enumerate(split128(gp.dh))] +
                                [(gp.rz[g]['wx'][mi][ki], xap)
                                 for ki, (xap, _) in enumerate(x_chunks)])
                        for ii, (wb, rhs) in enumerate(blks):
                            nc.tensor.matmul(ps[0:ms, :], WB(wb), rhs,
                                             start=(ii == 0),
                                             stop=(ii == len(blks) - 1))
                        nc.scalar.activation(gt[0:ms, mc], ps[0:ms, :],
                                             AF.Sigmoid,
                                             bias=BB(gp.rz[g]['b'][mi]))
                    ps_hn = pspool.tile([128, BT], F32, tag="ps")
                    hsp = split128(gp.dh)
                    for ki, ksz in enumerate(hsp):
                        nc.tensor.matmul(ps_hn[0:ms, :], WB(gp.nh[mi][ki]),
                                         h_tile[0:ksz, ki * BT:ki * BT + BT],
                                         start=(ki == 0),
                                         stop=(ki == len(hsp) - 1))
                    ps_in = pspool.tile([128, BT], F32, tag="ps")
                    for ki, (xap, _) in enumerate(x_chunks):
                        nc.tensor.matmul(ps_in[0:ms, :], WB(gp.nx[mi][ki]),
                                         xap, start=(ki == 0),
                                         stop=(ki == len(x_chunks) - 1))
                    # rhn = (hn + bhh_n) * r ; npre = (inn + bih_n) + rhn
                    nc.vector.scalar_tensor_tensor(
                        sc[0:ms, mc], ps_hn[0:ms, :], BB(gp.bnh[mi]),
                        r_t[0:ms, mc], op0=ALU.add, op1=ALU.mult)
                    nc.vector.scalar_tensor_tensor(
                        sc[0:ms, mc], ps_in[0:ms, :], BB(gp.bnx[mi]),
                        sc[0:ms, mc], op0=ALU.add, op1=ALU.add)
                # wide ops over all chunks at once
                nc.scalar.activation(n_t[:, :], sc[:, :], AF.Tanh)
                nc.vector.tensor_sub(sc[:, :], h_tile[:, 0:W_], n_t[:, :])
                nc.vector.tensor_mul(sc[:, :], z_t[:, :], sc[:, :])
                nc.vector.tensor_add(hout[:, :], sc[:, :], n_t[:, :])
                chunks = [(hout[0:ms, mi * BT:(mi + 1) * BT], ms)
                          for mi, ms in enumerate(gp.m_splits)]
                return chunks, hout

            def load_fm(nm, f, t, tag):
                sp = split128(f)
                tl = io.tile([128, BT * len(sp)], BF16, tag=tag)
                chunks = []
                fo = 0
                for ci, cs in enumerate(sp):
                    ap = tl[0:cs, ci * BT:(ci + 1) * BT]
                    nc.sync.dma_start(
                        ap, d_in[nm][fo:fo + cs, t * BT:(t + 1) * BT])
                    chunks.append((ap, cs))
                    fo += cs
                return chunks, tl

            def store_fm(nm, chunks, t):
                fo = 0
                for ap, cs in chunks:
                    nc.sync.dma_start(
                        d_out[nm][fo:fo + cs, t * BT:(t + 1) * BT], ap)
                    fo += cs

            def cast16(chunks, tag):
                n = len(chunks)
                ct = act.tile([128, BT * n], BF16, tag=tag)
                out = []
                for mi, (ap, ms) in enumerate(chunks):
                    cap = ct[0:ms, mi * BT:(mi + 1) * BT]
                    nc.gpsimd.tensor_copy(cap, ap)
                    out.append((cap, ms))
                return out

            inp = {}

            def emit_inputs(t):
                f12, _ = load_fm('f12', 24, t, 'f12')
                f3, _ = load_fm('f3', 16, t, 'f3')
                f4, _ = load_fm('f4', 16, t, 'f4')
                qh = load_fm('qh', QD, t, 'qh')
                ph = load_fm('ph', PD, t, 'ph')
                sh = load_fm('sh', SD, t, 'sh')
                inp[t] = {
                    'qh': qh, 'ph': ph, 'sh': sh,
                    'qinp': run_mlp('QI', f4, 'act'),
                    'pinp': run_mlp('PI', f3, 'act'),
                    'sinp': run_mlp('SI', f12, 'act'),
                }

            emit_inputs(0)
            for t in range(NT):
                cur = inp.pop(t)
                q, _ = run_gru('Q_GRU', cur['qinp'], cur['qh'][1], 'qout', F32)
                store_fm('q', q, t)
                q16 = cast16(q, 'q16')

                if t + 1 < NT:
                    emit_inputs(t + 1)

                p1, _ = run_gru('P_GRU', q16 + cur['pinp'], cur['ph'][1],
                                'pout', BF16)
                psx = run_mlp('PS', p1, 'act')
                s, _ = run_gru('S_GRU', cur['sinp'] + psx, cur['sh'][1],
                               'sout', F32)
                store_fm('s', s, t)
                s16 = cast16(s, 's16')

                kg = run_mlp('KG', s16 + p1, 'dve', final_f32=True)
                store_fm('kg', kg, t)
                kg16 = cast16(kg, 'kg16')

                sk = run_mlp('SK', s16 + kg16, 'act')
                p2 = run_mlp('PU', p1 + sk, 'dve', final_f32=True)
                store_fm('p', p2, t)

    nc.compile()
    return nc


def _get_nc():
    if 'nc' not in _COMPILED:
        _COMPILED['nc'] = _build()
    return _COMPILED['nc']


# ---------------------------------------------------------------------------
# Host entry point
# ---------------------------------------------------------------------------
def kernel(F1, F2, F3, F4, Q_GRU_HIDDEN, P_GRU_HIDDEN, S_GRU_HIDDEN, params,
           _want_results_obj=False, _trace=False):
    import ml_dtypes
    from concourse.bass_utils import run_bass_kernel_spmd

    BF = ml_dtypes.bfloat16
    F1 = np.asarray(F1, dtype=np.float32)
    F2 = np.asarray(F2, dtype=np.float32)
    F3 = np.asarray(F3, dtype=np.float32)
    F4 = np.asarray(F4, dtype=np.float32)
    QH = np.asarray(Q_GRU_HIDDEN, dtype=np.float32)
    PH = np.asarray(P_GRU_HIDDEN, dtype=np.float32)
    SH = np.asarray(S_GRU_HIDDEN, dtype=np.float32)

    f12T = np.empty((24, B), dtype=BF)
    f12T[0:12] = F1.T
    f12T[12:24] = F2.T
    f3T = F3.T.astype(BF)
    f4T = F4.T.astype(BF)
    qhT = QH.T.astype(BF)
    phT = PH.T.astype(BF)
    shT = SH.T.astype(BF)

    W, Bv = pack_weights(params)

    nc = _get_nc()
    in_maps = []
    for c in range(N_CORES):
        sl = slice(c * ROWS, (c + 1) * ROWS)
        in_maps.append({
            'f12': np.ascontiguousarray(f12T[:, sl]),
            'f3': np.ascontiguousarray(f3T[:, sl]),
            'f4': np.ascontiguousarray(f4T[:, sl]),
            'qh': np.ascontiguousarray(qhT[:, sl]),
            'ph': np.ascontiguousarray(phT[:, sl]),
            'sh': np.ascontiguousarray(shT[:, sl]),
            'wpack': W,
            'bpack': Bv,
        })
    res = run_bass_kernel_spmd(nc, in_maps, core_ids=list(range(N_CORES)),
                               trace=_trace)

    kgT = np.empty((192, B), dtype=np.float32)
    qT = np.empty((QD, B), dtype=np.float32)
    pT = np.empty((PD, B), dtype=np.float32)
    sT = np.empty((SD, B), dtype=np.float32)
    for c in range(N_CORES):
        sl = slice(c * ROWS, (c + 1) * ROWS)
        kgT[:, sl] = res.results[c]['kg']
        qT[:, sl] = res.results[c]['q']
        pT[:, sl] = res.results[c]['p']
        sT[:, sl] = res.results[c]['s']

    out = (np.ascontiguousarray(kgT.T).reshape(B, DS, DM),
           np.ascontiguousarray(qT.T),
           np.ascontiguousarray(pT.T),
           np.ascontiguousarray(sT.T))
    if _want_results_obj:
        return out, res
    return out
